# revision 8
# baseline (speedup 1.0000x reference)
"""Causal self-attention (B=2, T=4096, C=768, H=12, D=64) on 8 Trainium2 cores.

Sharding: (batch, head-group) across 8 cores — core i handles batch i//4,
heads 3*(i%4) .. 3*(i%4)+2.  Each core computes q/k in a transposed [d, T]
layout (S^T formulation: no transposes anywhere in attention), v in natural
[T, d] layout packed next to a ones-block so one AV matmul produces both
y_un^T and the broadcast softmax denominator.  Output projection produces a
partial z[T, C] per core; host sums the 4 partials per batch and adds biases.

Numerics: all matmuls run in float32r (TF32-class, fp32 accumulate), softmax
exp without max-subtraction (logits are O(10); fp32 exp is exact enough).
The v-bias and output bias are folded into a single host-side row added at
gather time: y @ W_p + b_p  ==  (y0/rowsum) @ W_p + (b_v @ W_p + b_p).
"""
import os
import sys

sys.path.insert(0, "/opt/trn_rl_repo")

import numpy as np

B, T, C = 2, 4096, 768
H, D = 12, 64
HPC = 3            # heads per core
NCORE = 8
QC = 512           # q-chunk (free dim of S^T blocks)
KTS = 128          # k-tile size
NJQ = T // QC      # 8 q-chunks
NKT = T // KTS     # 32 k-tiles
NTT = T // 128     # 32 t-tiles (proj)
NCCH = C // 128    # 6 contraction chunks

# vones column layout: [v0 | ones | v1 | v2 | ones]
VONES_W = 320
V_LHST = [0, 64, 192]    # lhsT col offset per local head ([V|1], [1|V], [V|1])
V_DST = [0, 128, 192]    # where phase A writes each head's v block
# q/k chunk + partition base per local head (bases must match between lhsT/rhs)
QKLOC = [(0, 0), (0, 64), (1, 0)]

_cache = {}
last_results = None  # set by kernel(); test.py reads exec_time_ns off this


def _build():
    import concourse.mybir as mybir
    import concourse.tile as tile
    from concourse import bacc

    F32 = mybir.dt.float32
    F32R = mybir.dt.float32r
    AF = mybir.ActivationFunctionType

    nc = bacc.Bacc("TRN2", target_bir_lowering=False, debug=False)

    xT = nc.dram_tensor("xT", [C, T], F32R, kind="ExternalInput").ap()
    wqk = nc.dram_tensor("wqk", [C, 384], F32R, kind="ExternalInput").ap()
    wv = nc.dram_tensor("wv", [C, 192], F32R, kind="ExternalInput").ap()
    wp = nc.dram_tensor("wp", [192, C], F32R, kind="ExternalInput").ap()
    bqk = nc.dram_tensor("bqk", [128, 4], F32, kind="ExternalInput").ap()
    trimask = nc.dram_tensor("trimask", [128, 128], F32R, kind="ExternalInput").ap()
    z = nc.dram_tensor("z", [T, C], F32, kind="ExternalOutput").ap()
    debug = os.environ.get("CC_ATTN_DEBUG", "0") == "1"
    if debug:
        dq = nc.dram_tensor("dbg_qT", [128, 2, T], mybir.dt.float32, kind="ExternalOutput").ap()
        dk = nc.dram_tensor("dbg_kT", [128, 2, T], mybir.dt.float32, kind="ExternalOutput").ap()
        dv = nc.dram_tensor("dbg_vones", [128, 32 * VONES_W], mybir.dt.float32, kind="ExternalOutput").ap()
        dy0 = nc.dram_tensor("dbg_yT0", [128, T], mybir.dt.float32, kind="ExternalOutput").ap()
        dy1 = nc.dram_tensor("dbg_yT1", [64, T], mybir.dt.float32, kind="ExternalOutput").ap()

    with tile.TileContext(nc) as tc:
        with tc.tile_pool(name="persist", bufs=1) as persist:
            qT = persist.tile([128, 2, T], F32R, tag="qT")
            kT = persist.tile([128, 2, T], F32R, tag="kT")
            vones = persist.tile([128, NKT, VONES_W], F32R, tag="vones")
            yT0 = persist.tile([128, T], F32R, tag="yT0")
            yT1 = persist.tile([64, T], F32R, tag="yT1")
            bqk_sb = persist.tile([128, 4], F32, tag="bqk")
            tri_sb = persist.tile([128, 128], F32R, tag="tri")

            nc.sync.dma_start(bqk_sb[:], bqk)
            nc.sync.dma_start(tri_sb[:], trimask)
            nc.vector.memset(vones[:].bitcast(mybir.dt.float32), 1.0)

            # ---------------- Phase A: qkv projections ----------------
            with (
                nc.named_scope("phaseA"),
                tc.tile_pool(name="aw", bufs=1) as aw,
                tc.tile_pool(name="ax", bufs=3) as ax,
                tc.tile_pool(name="apsA", bufs=4, space="PSUM") as apsA,
                tc.tile_pool(name="apsV", bufs=4, space="PSUM") as apsV,
            ):
                wqk_sb = aw.tile([128, NCCH, 384], F32R, tag="wqk")
                wv_sb = aw.tile([128, NCCH, 192], F32R, tag="wv")
                nc.sync.dma_start(wqk_sb[:], wqk.rearrange("(ko p) m -> p ko m", p=128))
                nc.sync.dma_start(wv_sb[:], wv.rearrange("(ko p) m -> p ko m", p=128))

                for tch in range(NJQ):
                    tcols = slice(tch * QC, (tch + 1) * QC)
                    psA = [apsA.tile([128, QC], F32, tag="psA", name=f"psA{i}")
                           for i in range(4)]
                    psV = [apsV.tile([128, 192], F32, tag="psV", name=f"psV{i}")
                           for i in range(4)]
                    for cch in range(NCCH):
                        xt = ax.tile([128, QC], F32R, tag="xt")
                        nc.sync.dma_start(
                            xt[:], xT[cch * 128:(cch + 1) * 128, tcols])
                        st, sp = (cch == 0), (cch == NCCH - 1)
                        nc.tensor.matmul(psA[0][:], wqk_sb[:, cch, 0:128], xt[:],
                                         start=st, stop=sp)
                        nc.tensor.matmul(psA[1][0:64], wqk_sb[:, cch, 128:192], xt[:],
                                         start=st, stop=sp)
                        nc.tensor.matmul(psA[2][:], wqk_sb[:, cch, 192:320], xt[:],
                                         start=st, stop=sp)
                        nc.tensor.matmul(psA[3][0:64], wqk_sb[:, cch, 320:384], xt[:],
                                         start=st, stop=sp)
                        for sub in range(4):
                            nc.tensor.matmul(
                                psV[sub][:],
                                xt[:, sub * 128:(sub + 1) * 128],
                                wv_sb[:, cch, :],
                                start=st, stop=sp)
                    # evacuate: q/k with bias, v into vones
                    nc.vector.tensor_scalar_add(qT[:, 0, tcols], psA[0][:],
                                                bqk_sb[:, 0:1])
                    nc.vector.tensor_scalar_add(qT[0:64, 1, tcols], psA[1][0:64],
                                                bqk_sb[0:64, 1:2])
                    nc.vector.tensor_scalar_add(kT[:, 0, tcols], psA[2][:],
                                                bqk_sb[:, 2:3])
                    nc.vector.tensor_scalar_add(kT[0:64, 1, tcols], psA[3][0:64],
                                                bqk_sb[0:64, 3:4])
                    for sub in range(4):
                        tt = tch * 4 + sub
                        for h in range(HPC):
                            nc.vector.tensor_copy(
                                vones[:, tt, V_DST[h]:V_DST[h] + 64],
                                psV[sub][:, h * 64:(h + 1) * 64])

            # ---------------- Phase B: attention ----------------
            with (
                nc.named_scope("phaseB"),
                tc.tile_pool(name="bpsS", bufs=3, space="PSUM") as bpsS,
                tc.tile_pool(name="bpsY", bufs=2, space="PSUM") as bpsY,
                tc.tile_pool(name="bexp", bufs=3) as bexp,
                tc.tile_pool(name="bst", bufs=4) as bst,
            ):
                for jq in range(NJQ):
                    for h in range(HPC):
                        qc, qb = QKLOC[h]
                        kq = qT[qb:qb + 64, qc, jq * QC:(jq + 1) * QC]
                        # units: pairs of full blocks, then 4 diagonal singles
                        units = [(2 * p, 2 * p + 1) for p in range(2 * jq)]
                        units += [(4 * jq + r,) for r in range(4)]
                        nu = len(units)
                        es_tiles = [None] * nu
                        psY = bpsY.tile([128, QC], F32, tag="psY")

                        def emit_S(ui):
                            u = units[ui]
                            ps = bpsS.tile([128, 1024], F32, tag="psS")
                            es = bexp.tile([128, 1024], F32R, tag="es")
                            kc, kb = QKLOC[h]
                            if len(u) == 2:
                                for j, kt in enumerate(u):
                                    nc.tensor.matmul(
                                        ps[:, j * QC:(j + 1) * QC],
                                        kT[kb:kb + 64, kc, kt * KTS:(kt + 1) * KTS],
                                        kq, start=True, stop=True)
                                nc.scalar.activation(es[:, 0:1024], ps[:, 0:1024],
                                                     AF.Exp)
                            else:
                                kt = u[0]
                                r = kt - 4 * jq
                                off = r * KTS
                                w = QC - off
                                nc.tensor.matmul(
                                    ps[:, 0:w],
                                    kT[kb:kb + 64, kc, kt * KTS:(kt + 1) * KTS],
                                    qT[qb:qb + 64, qc,
                                       jq * QC + off:(jq + 1) * QC],
                                    start=True, stop=True)
                                nc.scalar.activation(es[:, 0:w], ps[:, 0:w], AF.Exp)
                                # straddle block: zero strictly-lower triangle
                                nc.vector.tensor_mul(es[:, 0:128], es[:, 0:128],
                                                     tri_sb[:])
                            es_tiles[ui] = es

                        def emit_AV(ui):
                            u = units[ui]
                            es = es_tiles[ui]
                            first = (ui == 0)
                            last = (ui == nu - 1)
                            vc = V_LHST[h]
                            if len(u) == 2:
                                for j, kt in enumerate(u):
                                    nc.tensor.matmul(
                                        psY[:],
                                        vones[:, kt, vc:vc + 128],
                                        es[:, j * QC:(j + 1) * QC],
                                        start=(first and j == 0),
                                        stop=(last and j == 1))
                            else:
                                kt = u[0]
                                r = kt - 4 * jq
                                off = r * KTS
                                w = QC - off
                                nc.tensor.matmul(
                                    psY[:, off:QC],
                                    vones[:, kt, vc:vc + 128],
                                    es[:, 0:w],
                                    start=first, stop=last)
                            es_tiles[ui] = None

                        emit_S(0)
                        if nu > 1:
                            emit_S(1)
                        for ui in range(nu):
                            if ui + 2 < nu:
                                emit_S(ui + 2)
                            emit_AV(ui)

                        # normalize: y = y_un / rowsum  (rowsum 64x-replicated
                        # on the complementary partition half)
                        st = bst.tile([128, QC], F32, tag="st")
                        rt = bst.tile([128, QC], F32, tag="rt")
                        ycols = slice(jq * QC, (jq + 1) * QC)
                        if h == 1:
                            nc.vector.reciprocal(st[0:64, :], psY[0:64, :])
                            nc.sync.dma_start(rt[64:128, :], st[0:64, :])
                            nc.vector.tensor_mul(yT0[64:128, ycols],
                                                 psY[64:128, :], rt[64:128, :])
                        else:
                            nc.vector.reciprocal(st[64:128, :], psY[64:128, :])
                            nc.sync.dma_start(rt[0:64, :], st[64:128, :])
                            dst = yT0[0:64, ycols] if h == 0 else yT1[:, ycols]
                            nc.vector.tensor_mul(dst, psY[0:64, :], rt[0:64, :])

            if debug:
                nc.sync.dma_start(dq, qT[:].bitcast(mybir.dt.float32))
                nc.sync.dma_start(dk, kT[:].bitcast(mybir.dt.float32))
                nc.sync.dma_start(dv, vones[:].bitcast(mybir.dt.float32))
                nc.sync.dma_start(dy0, yT0[:].bitcast(mybir.dt.float32))
                nc.sync.dma_start(dy1, yT1[:].bitcast(mybir.dt.float32))

            # ---------------- Phase D: output projection ----------------
            with (
                nc.named_scope("phaseD"),
                tc.tile_pool(name="dw", bufs=1) as dw,
                tc.tile_pool(name="dz", bufs=3) as dz,
                tc.tile_pool(name="dps", bufs=3, space="PSUM") as dps,
            ):
                wp0_sb = dw.tile([128, C], F32R, tag="wp0")
                wp1_sb = dw.tile([64, C], F32R, tag="wp1")
                nc.sync.dma_start(wp0_sb[:], wp[0:128, :])
                nc.sync.dma_start(wp1_sb[:], wp[128:192, :])

                for tt in range(NTT):
                    tsl = slice(tt * 128, (tt + 1) * 128)
                    pz = dps.tile([128, C], F32, tag="pz")
                    for ncol, (a, b) in enumerate(((0, 512), (512, 768))):
                        nc.tensor.matmul(pz[:, a:b], yT0[:, tsl], wp0_sb[:, a:b],
                                         start=True, stop=False)
                        nc.tensor.matmul(pz[:, a:b], yT1[:, tsl], wp1_sb[:, a:b],
                                         start=False, stop=True)
                    zt = dz.tile([128, C], F32, tag="zt")
                    nc.vector.tensor_copy(zt[:], pz[:])
                    nc.sync.dma_start(z[tsl, :], zt[:])

    nc.compile()
    return nc


def _get_program():
    if "nc" not in _cache:
        _cache["nc"] = _build()
    return _cache["nc"]


def kernel(x, W_attn, b_attn, W_proj, b_proj):
    global last_results
    from concourse.bass_utils import run_bass_kernel_spmd

    x = np.asarray(x, dtype=np.float32)
    W_attn = np.asarray(W_attn, dtype=np.float32)
    b_attn = np.asarray(b_attn, dtype=np.float32)
    W_proj = np.asarray(W_proj, dtype=np.float32)
    b_proj = np.asarray(b_proj, dtype=np.float32)

    Wq, Wk, Wv = W_attn[:, 0:C], W_attn[:, C:2 * C], W_attn[:, 2 * C:3 * C]
    bq, bk, bv = b_attn[0:C], b_attn[C:2 * C], b_attn[2 * C:3 * C]
    scale = 1.0 / np.sqrt(D)

    xTb = [np.ascontiguousarray(x[b].T) for b in range(B)]
    tri = np.triu(np.ones((128, 128), dtype=np.float32))  # keep f >= p

    in_maps = []
    for core in range(NCORE):
        b = core // 4
        h0 = 3 * (core % 4)
        cs = slice(h0 * D, (h0 + HPC) * D)  # this core's 192 channels
        wqk_i = np.concatenate(
            [Wq[:, cs] * scale, Wk[:, cs]], axis=1)  # [768, 384]
        bqk_i = np.zeros((128, 4), dtype=np.float32)
        bqk_i[:, 0] = bq[cs][0:128] * scale
        bqk_i[0:64, 1] = bq[cs][128:192] * scale
        bqk_i[:, 2] = bk[cs][0:128]
        bqk_i[0:64, 3] = bk[cs][128:192]
        in_maps.append({
            "xT": xTb[b],
            "wqk": np.ascontiguousarray(wqk_i),
            "wv": np.ascontiguousarray(Wv[:, cs]),
            "wp": np.ascontiguousarray(W_proj[cs, :]),
            "bqk": bqk_i,
            "trimask": tri,
        })

    nc = _get_program()
    trace = os.environ.get("CC_ATTN_TRACE", "0") == "1"
    res = run_bass_kernel_spmd(nc, in_maps, core_ids=list(range(NCORE)),
                               trace=trace)
    last_results = res

    bias_row = (b_proj + bv @ W_proj).astype(np.float32)  # [768]
    out = np.empty((B, T, C), dtype=np.float32)
    for b in range(B):
        acc = res.results[4 * b]["z"].astype(np.float32).copy()
        for g in range(1, 4):
            acc += res.results[4 * b + g]["z"]
        out[b] = acc + bias_row
    return out


# revision 10
# speedup vs baseline: 1.4168x; 1.4168x over previous
"""Causal self-attention (B=2, T=4096, C=768, H=12, D=64) on 8 Trainium2 cores.

Sharding: (batch, head-group) across 8 cores — core i handles batch i//4,
heads 3*(i%4) .. 3*(i%4)+2.  Each core computes q/k in a transposed [d, T]
layout (S^T formulation: no transposes anywhere in attention), v in natural
[T, d] layout packed next to a ones-block so one AV matmul produces both
y_un^T and the broadcast softmax denominator.  Output projection produces a
partial z[T, C] per core; host sums the 4 partials per batch and adds biases.

Numerics: all matmuls run in float32r (TF32-class, fp32 accumulate), softmax
exp without max-subtraction (logits are O(10); fp32 exp is exact enough).
The v-bias and output bias are folded into a single host-side row added at
gather time: y @ W_p + b_p  ==  (y0/rowsum) @ W_p + (b_v @ W_p + b_p).
"""
import os
import sys

sys.path.insert(0, "/opt/trn_rl_repo")

import numpy as np

B, T, C = 2, 4096, 768
H, D = 12, 64
HPC = 3            # heads per core
NCORE = 8
QC = 512           # q-chunk (free dim of S^T blocks)
KTS = 128          # k-tile size
NJQ = T // QC      # 8 q-chunks
NKT = T // KTS     # 32 k-tiles
NTT = T // 128     # 32 t-tiles (proj)
NCCH = C // 128    # 6 contraction chunks

# vones column layout: [v0 | ones | v1 | v2 | ones]
VONES_W = 320
V_LHST = [0, 64, 192]    # lhsT col offset per local head ([V|1], [1|V], [V|1])
V_DST = [0, 128, 192]    # where phase A writes each head's v block
# q/k chunk + partition base per local head (bases must match between lhsT/rhs)
QKLOC = [(0, 0), (0, 64), (1, 0)]

_cache = {}
last_results = None  # set by kernel(); test.py reads exec_time_ns off this


def _build():
    import concourse.mybir as mybir
    import concourse.tile as tile
    from concourse import bacc

    F32 = mybir.dt.float32
    F16 = mybir.dt.float16
    AF = mybir.ActivationFunctionType

    nc = bacc.Bacc("TRN2", target_bir_lowering=False, debug=False)

    xT = nc.dram_tensor("xT", [C, T], F16, kind="ExternalInput").ap()
    wqk = nc.dram_tensor("wqk", [C, 384], F16, kind="ExternalInput").ap()
    wv = nc.dram_tensor("wv", [C, 192], F16, kind="ExternalInput").ap()
    wp = nc.dram_tensor("wp", [192, C], F16, kind="ExternalInput").ap()
    bqk = nc.dram_tensor("bqk", [128, 4], F32, kind="ExternalInput").ap()
    trimask = nc.dram_tensor("trimask", [128, 128], F16, kind="ExternalInput").ap()
    z = nc.dram_tensor("z", [T, C], F32, kind="ExternalOutput").ap()
    debug = os.environ.get("CC_ATTN_DEBUG", "0") == "1"
    if debug:
        dq = nc.dram_tensor("dbg_qT", [128, 2, T], mybir.dt.float16, kind="ExternalOutput").ap()
        dk = nc.dram_tensor("dbg_kT", [128, 2, T], mybir.dt.float16, kind="ExternalOutput").ap()
        dv = nc.dram_tensor("dbg_vones", [128, 32 * VONES_W], mybir.dt.float16, kind="ExternalOutput").ap()
        dy0 = nc.dram_tensor("dbg_yT0", [128, T], mybir.dt.float16, kind="ExternalOutput").ap()
        dy1 = nc.dram_tensor("dbg_yT1", [64, T], mybir.dt.float16, kind="ExternalOutput").ap()

    with tile.TileContext(nc) as tc:
        with tc.tile_pool(name="persist", bufs=1) as persist:
            qT = persist.tile([128, 2, T], F16, tag="qT")
            kT = persist.tile([128, 2, T], F16, tag="kT")
            vones = persist.tile([128, NKT, VONES_W], F16, tag="vones")
            yT0 = persist.tile([128, T], F16, tag="yT0")
            yT1 = persist.tile([64, T], F16, tag="yT1")
            bqk_sb = persist.tile([128, 4], F32, tag="bqk")
            shift_sb = persist.tile([128, 1], F32, tag="shift")
            tri_sb = persist.tile([128, 128], F16, tag="tri")

            nc.sync.dma_start(bqk_sb[:], bqk)
            nc.vector.memset(shift_sb[:], -10.0)
            nc.sync.dma_start(tri_sb[:], trimask)
            nc.vector.memset(vones[:], 1.0)

            # ---------------- Phase A: qkv projections ----------------
            with (
                nc.named_scope("phaseA"),
                tc.tile_pool(name="aw", bufs=1) as aw,
                tc.tile_pool(name="ax", bufs=3) as ax,
                tc.tile_pool(name="apsA", bufs=4, space="PSUM") as apsA,
                tc.tile_pool(name="apsV", bufs=4, space="PSUM") as apsV,
            ):
                wqk_sb = aw.tile([128, NCCH, 384], F16, tag="wqk")
                wv_sb = aw.tile([128, NCCH, 192], F16, tag="wv")
                nc.sync.dma_start(wqk_sb[:], wqk.rearrange("(ko p) m -> p ko m", p=128))
                nc.sync.dma_start(wv_sb[:], wv.rearrange("(ko p) m -> p ko m", p=128))

                for tch in range(NJQ):
                    tcols = slice(tch * QC, (tch + 1) * QC)
                    psA = [apsA.tile([128, QC], F32, tag="psA", name=f"psA{i}")
                           for i in range(4)]
                    psV = [apsV.tile([128, 192], F32, tag="psV", name=f"psV{i}")
                           for i in range(4)]
                    for cch in range(NCCH):
                        xt = ax.tile([128, QC], F16, tag="xt")
                        nc.sync.dma_start(
                            xt[:], xT[cch * 128:(cch + 1) * 128, tcols])
                        st, sp = (cch == 0), (cch == NCCH - 1)
                        nc.tensor.matmul(psA[0][:], wqk_sb[:, cch, 0:128], xt[:],
                                         start=st, stop=sp)
                        nc.tensor.matmul(psA[1][0:64], wqk_sb[:, cch, 128:192], xt[:],
                                         start=st, stop=sp)
                        nc.tensor.matmul(psA[2][:], wqk_sb[:, cch, 192:320], xt[:],
                                         start=st, stop=sp)
                        nc.tensor.matmul(psA[3][0:64], wqk_sb[:, cch, 320:384], xt[:],
                                         start=st, stop=sp)
                        for sub in range(4):
                            nc.tensor.matmul(
                                psV[sub][:],
                                xt[:, sub * 128:(sub + 1) * 128],
                                wv_sb[:, cch, :],
                                start=st, stop=sp)
                    # evacuate: q/k with bias, v into vones
                    nc.vector.tensor_scalar_add(qT[:, 0, tcols], psA[0][:],
                                                bqk_sb[:, 0:1])
                    nc.vector.tensor_scalar_add(qT[0:64, 1, tcols], psA[1][0:64],
                                                bqk_sb[0:64, 1:2])
                    nc.vector.tensor_scalar_add(kT[:, 0, tcols], psA[2][:],
                                                bqk_sb[:, 2:3])
                    nc.vector.tensor_scalar_add(kT[0:64, 1, tcols], psA[3][0:64],
                                                bqk_sb[0:64, 3:4])
                    for sub in range(4):
                        tt = tch * 4 + sub
                        for h in range(HPC):
                            nc.vector.tensor_copy(
                                vones[:, tt, V_DST[h]:V_DST[h] + 64],
                                psV[sub][:, h * 64:(h + 1) * 64])

            # ---------------- Phase B: attention ----------------
            with (
                nc.named_scope("phaseB"),
                tc.tile_pool(name="bpsS", bufs=3, space="PSUM") as bpsS,
                tc.tile_pool(name="bpsY", bufs=2, space="PSUM") as bpsY,
                tc.tile_pool(name="bexp", bufs=3) as bexp,
                tc.tile_pool(name="bst", bufs=4) as bst,
            ):
                for jq in range(NJQ):
                    for h in range(HPC):
                        qc, qb = QKLOC[h]
                        kq = qT[qb:qb + 64, qc, jq * QC:(jq + 1) * QC]
                        # units: pairs of full blocks, then 4 diagonal singles
                        units = [(2 * p, 2 * p + 1) for p in range(2 * jq)]
                        units += [(4 * jq + r,) for r in range(4)]
                        nu = len(units)
                        es_tiles = [None] * nu
                        psY = bpsY.tile([128, QC], F32, tag="psY")

                        def emit_S(ui):
                            u = units[ui]
                            ps = bpsS.tile([128, 1024], F32, tag="psS")
                            es = bexp.tile([128, 1024], F16, tag="es")
                            kc, kb = QKLOC[h]
                            if len(u) == 2:
                                for j, kt in enumerate(u):
                                    nc.tensor.matmul(
                                        ps[:, j * QC:(j + 1) * QC],
                                        kT[kb:kb + 64, kc, kt * KTS:(kt + 1) * KTS],
                                        kq, start=True, stop=True)
                                nc.scalar.activation(es[:, 0:1024], ps[:, 0:1024],
                                                     AF.Exp, bias=shift_sb[:, 0:1])
                            else:
                                kt = u[0]
                                r = kt - 4 * jq
                                off = r * KTS
                                w = QC - off
                                nc.tensor.matmul(
                                    ps[:, 0:w],
                                    kT[kb:kb + 64, kc, kt * KTS:(kt + 1) * KTS],
                                    qT[qb:qb + 64, qc,
                                       jq * QC + off:(jq + 1) * QC],
                                    start=True, stop=True)
                                nc.scalar.activation(es[:, 0:w], ps[:, 0:w], AF.Exp, bias=shift_sb[:, 0:1])
                                # straddle block: zero strictly-lower triangle
                                nc.vector.tensor_mul(es[:, 0:128], es[:, 0:128],
                                                     tri_sb[:])
                            es_tiles[ui] = es

                        def emit_AV(ui):
                            u = units[ui]
                            es = es_tiles[ui]
                            first = (ui == 0)
                            last = (ui == nu - 1)
                            vc = V_LHST[h]
                            if len(u) == 2:
                                for j, kt in enumerate(u):
                                    nc.tensor.matmul(
                                        psY[:],
                                        vones[:, kt, vc:vc + 128],
                                        es[:, j * QC:(j + 1) * QC],
                                        start=(first and j == 0),
                                        stop=(last and j == 1))
                            else:
                                kt = u[0]
                                r = kt - 4 * jq
                                off = r * KTS
                                w = QC - off
                                nc.tensor.matmul(
                                    psY[:, off:QC],
                                    vones[:, kt, vc:vc + 128],
                                    es[:, 0:w],
                                    start=first, stop=last)
                            es_tiles[ui] = None

                        emit_S(0)
                        if nu > 1:
                            emit_S(1)
                        for ui in range(nu):
                            if ui + 2 < nu:
                                emit_S(ui + 2)
                            emit_AV(ui)

                        # normalize: y = y_un / rowsum  (rowsum 64x-replicated
                        # on the complementary partition half)
                        st = bst.tile([128, QC], F32, tag="st")
                        rt = bst.tile([128, QC], F32, tag="rt")
                        ycols = slice(jq * QC, (jq + 1) * QC)
                        if h == 1:
                            nc.vector.reciprocal(st[0:64, :], psY[0:64, :])
                            nc.sync.dma_start(rt[64:128, :], st[0:64, :])
                            nc.vector.tensor_mul(yT0[64:128, ycols],
                                                 psY[64:128, :], rt[64:128, :])
                        else:
                            nc.vector.reciprocal(st[64:128, :], psY[64:128, :])
                            nc.sync.dma_start(rt[0:64, :], st[64:128, :])
                            dst = yT0[0:64, ycols] if h == 0 else yT1[:, ycols]
                            nc.vector.tensor_mul(dst, psY[0:64, :], rt[0:64, :])

            if debug:
                nc.sync.dma_start(dq, qT[:])
                nc.sync.dma_start(dk, kT[:])
                nc.sync.dma_start(dv, vones[:])
                nc.sync.dma_start(dy0, yT0[:])
                nc.sync.dma_start(dy1, yT1[:])

            # ---------------- Phase D: output projection ----------------
            with (
                nc.named_scope("phaseD"),
                tc.tile_pool(name="dw", bufs=1) as dw,
                tc.tile_pool(name="dz", bufs=3) as dz,
                tc.tile_pool(name="dps", bufs=3, space="PSUM") as dps,
            ):
                wp0_sb = dw.tile([128, C], F16, tag="wp0")
                wp1_sb = dw.tile([64, C], F16, tag="wp1")
                nc.sync.dma_start(wp0_sb[:], wp[0:128, :])
                nc.sync.dma_start(wp1_sb[:], wp[128:192, :])

                for tt in range(NTT):
                    tsl = slice(tt * 128, (tt + 1) * 128)
                    pz = dps.tile([128, C], F32, tag="pz")
                    for ncol, (a, b) in enumerate(((0, 512), (512, 768))):
                        nc.tensor.matmul(pz[:, a:b], yT0[:, tsl], wp0_sb[:, a:b],
                                         start=True, stop=False)
                        nc.tensor.matmul(pz[:, a:b], yT1[:, tsl], wp1_sb[:, a:b],
                                         start=False, stop=True)
                    zt = dz.tile([128, C], F32, tag="zt")
                    nc.vector.tensor_copy(zt[:], pz[:])
                    nc.sync.dma_start(z[tsl, :], zt[:])

    nc.compile()
    return nc


def _get_program():
    if "nc" not in _cache:
        _cache["nc"] = _build()
    return _cache["nc"]


def kernel(x, W_attn, b_attn, W_proj, b_proj):
    global last_results
    from concourse.bass_utils import run_bass_kernel_spmd

    x = np.asarray(x, dtype=np.float32)
    W_attn = np.asarray(W_attn, dtype=np.float32)
    b_attn = np.asarray(b_attn, dtype=np.float32)
    W_proj = np.asarray(W_proj, dtype=np.float32)
    b_proj = np.asarray(b_proj, dtype=np.float32)

    Wq, Wk, Wv = W_attn[:, 0:C], W_attn[:, C:2 * C], W_attn[:, 2 * C:3 * C]
    bq, bk, bv = b_attn[0:C], b_attn[C:2 * C], b_attn[2 * C:3 * C]
    scale = 1.0 / np.sqrt(D)

    xTb = [np.ascontiguousarray(x[b].T).astype(np.float16) for b in range(B)]
    tri = np.triu(np.ones((128, 128), dtype=np.float32))  # keep f >= p

    in_maps = []
    for core in range(NCORE):
        b = core // 4
        h0 = 3 * (core % 4)
        cs = slice(h0 * D, (h0 + HPC) * D)  # this core's 192 channels
        wqk_i = np.concatenate(
            [Wq[:, cs] * scale, Wk[:, cs]], axis=1)  # [768, 384]
        bqk_i = np.zeros((128, 4), dtype=np.float32)
        bqk_i[:, 0] = bq[cs][0:128] * scale
        bqk_i[0:64, 1] = bq[cs][128:192] * scale
        bqk_i[:, 2] = bk[cs][0:128]
        bqk_i[0:64, 3] = bk[cs][128:192]
        in_maps.append({
            "xT": xTb[b],
            "wqk": wqk_i.astype(np.float16),
            "wv": np.ascontiguousarray(Wv[:, cs]).astype(np.float16),
            "wp": np.ascontiguousarray(W_proj[cs, :]).astype(np.float16),
            "bqk": bqk_i,
            "trimask": tri.astype(np.float16),
        })

    nc = _get_program()
    trace = os.environ.get("CC_ATTN_TRACE", "0") == "1"
    res = run_bass_kernel_spmd(nc, in_maps, core_ids=list(range(NCORE)),
                               trace=trace)
    last_results = res

    bias_row = (b_proj + bv @ W_proj).astype(np.float32)  # [768]
    out = np.empty((B, T, C), dtype=np.float32)
    for b in range(B):
        acc = res.results[4 * b]["z"].astype(np.float32).copy()
        for g in range(1, 4):
            acc += res.results[4 * b + g]["z"]
        out[b] = acc + bias_row
    return out


# revision 11
# speedup vs baseline: 1.8839x; 1.3297x over previous
"""Causal self-attention (B=2, T=4096, C=768, H=12, D=64) on 8 Trainium2 cores.

Sharding: (batch, head-group) across 8 cores — core i handles batch i//4,
heads 3*(i%4) .. 3*(i%4)+2.  Each core computes q/k in a transposed [d, T]
layout (S^T formulation: no transposes anywhere in attention), v in natural
[T, d] layout packed next to a ones-block so one AV matmul produces both
y_un^T and the broadcast softmax denominator.  Output projection produces a
partial z[T, C] per core; host sums the 4 partials per batch and adds biases.

Numerics: all matmuls in fp16 (same 10-bit mantissa as TF32/fp32r, but full
PE rate), fp32 PSUM accumulation.  Softmax exp has no max-subtraction; a
constant exp(S-10) shift keeps probs inside fp16 range and cancels in the
normalization.  v-bias and output bias fold into one host-side row:
y @ W_p + b_p == (y0/rowsum) @ W_p + (b_v @ W_p + b_p).

Perf notes (measured on HW): fp16/bf16 matmul N=512 is ~222 ns warm; matmuls
that alternate partition base or contraction row-groups pay ~100 ns per
transition, so all heads keep base-0 operands and S-matmul (K=64) / AV-matmul
(K=128) runs are batched ~4-long.  DVE reciprocal is ~3.4 us per call, so the
normalization chain stages everything to SBUF to free PSUM immediately.
"""
import os
import sys

sys.path.insert(0, "/opt/trn_rl_repo")

import numpy as np

B, T, C = 2, 4096, 768
H, D = 12, 64
HPC = 3            # heads per core
NCORE = 8
QC = 512           # q-chunk (free dim of S^T blocks)
KTS = 128          # k-tile size
NJQ = T // QC      # 8 q-chunks
NKT = T // KTS     # 32 k-tiles
NTT = T // 128     # 32 t-tiles (proj)
NCCH = C // 128    # 6 contraction chunks

# vones column layout: [v0 | ones | v1 | v2 | ones]
VONES_W = 320
V_LHST = [0, 64, 192]    # lhsT col offset per local head ([V|1], [1|V], [V|1])
V_DST = [0, 128, 192]    # where phase A writes each head's v block
EXP_SHIFT = -10.0

_cache = {}
last_results = None  # set by kernel(); test.py reads exec_time_ns off this


def _build():
    import concourse.mybir as mybir
    import concourse.tile as tile
    from concourse import bacc

    F32 = mybir.dt.float32
    F16 = mybir.dt.float16
    AF = mybir.ActivationFunctionType

    nc = bacc.Bacc("TRN2", target_bir_lowering=False, debug=False)

    xT = nc.dram_tensor("xT", [C, T], F16, kind="ExternalInput").ap()
    wqk = nc.dram_tensor("wqk", [C, 384], F16, kind="ExternalInput").ap()
    wv = nc.dram_tensor("wv", [C, 192], F16, kind="ExternalInput").ap()
    wp = nc.dram_tensor("wp", [192, C], F16, kind="ExternalInput").ap()
    bqk = nc.dram_tensor("bqk", [128, 3], F32, kind="ExternalInput").ap()
    trimask = nc.dram_tensor("trimask", [128, 128], F16, kind="ExternalInput").ap()
    z = nc.dram_tensor("z", [T, C], F32, kind="ExternalOutput").ap()
    debug = os.environ.get("CC_ATTN_DEBUG", "0") == "1"
    if debug:
        dq = nc.dram_tensor("dbg_qT", [64, HPC, T], F16, kind="ExternalOutput").ap()
        dk = nc.dram_tensor("dbg_kT", [64, HPC, T], F16, kind="ExternalOutput").ap()
        dv = nc.dram_tensor("dbg_vones", [128, 32 * VONES_W], F16,
                            kind="ExternalOutput").ap()
        dy0 = nc.dram_tensor("dbg_yT0", [128, T], F16, kind="ExternalOutput").ap()
        dy1 = nc.dram_tensor("dbg_yT1", [64, T], F16, kind="ExternalOutput").ap()

    with tile.TileContext(nc) as tc:
        with tc.tile_pool(name="persist", bufs=1) as persist:
            qT = persist.tile([64, HPC, T], F16, tag="qT")
            kT = persist.tile([64, HPC, T], F16, tag="kT")
            vones = persist.tile([128, NKT, VONES_W], F16, tag="vones")
            yT0 = persist.tile([128, T], F16, tag="yT0")
            yT1 = persist.tile([64, T], F16, tag="yT1")
            bqk_sb = persist.tile([128, 3], F32, tag="bqk")
            shift_sb = persist.tile([128, 1], F32, tag="shift")
            tri_sb = persist.tile([128, 128], F16, tag="tri")

            nc.sync.dma_start(bqk_sb[:], bqk)
            nc.sync.dma_start(tri_sb[:], trimask)
            nc.vector.memset(shift_sb[:], EXP_SHIFT)
            nc.vector.memset(vones[:], 1.0)

            # ---------------- Phase A: qkv projections ----------------
            # wqk columns: [q0 q1 | k0 k1 | q2 k2]; psum rows 64:128 of each
            # m-tile land on the "wrong" partitions for their head and get
            # staged + partition-shift-DMA'd into place.
            with (
                nc.named_scope("phaseA"),
                tc.tile_pool(name="aw", bufs=1) as aw,
                tc.tile_pool(name="ax", bufs=2) as ax,
                tc.tile_pool(name="ast", bufs=3) as ast,
                tc.tile_pool(name="apsA", bufs=3, space="PSUM") as apsA,
                tc.tile_pool(name="apsV", bufs=4, space="PSUM") as apsV,
            ):
                wqk_sb = aw.tile([128, NCCH, 384], F16, tag="wqk")
                wv_sb = aw.tile([128, NCCH, 192], F16, tag="wv")
                nc.sync.dma_start(wqk_sb[:], wqk.rearrange("(ko p) m -> p ko m", p=128))
                nc.sync.dma_start(wv_sb[:], wv.rearrange("(ko p) m -> p ko m", p=128))

                for tch in range(NJQ):
                    tcols = slice(tch * QC, (tch + 1) * QC)
                    xslab = ax.tile([128, NCCH, QC], F16, tag="xslab")
                    nc.sync.dma_start(
                        xslab[:], xT[:, tcols].rearrange("(ko p) t -> p ko t", p=128))
                    # m-tiles: 0 -> (q0, q1), 1 -> (k0, k1), 2 -> (q2, k2)
                    for mt in range(3):
                        ps = apsA.tile([128, QC], F32, tag="psA", name=f"psA{tch}_{mt}")
                        for cch in range(NCCH):
                            nc.tensor.matmul(
                                ps[:], wqk_sb[:, cch, mt * 128:(mt + 1) * 128],
                                xslab[:, cch, :],
                                start=(cch == 0), stop=(cch == NCCH - 1))
                        lo_dst = [qT[0:64, 0, tcols], kT[0:64, 0, tcols],
                                  qT[0:64, 2, tcols]][mt]
                        hi_dst = [qT[0:64, 1, tcols], kT[0:64, 1, tcols],
                                  kT[0:64, 2, tcols]][mt]
                        nc.vector.tensor_scalar_add(lo_dst, ps[0:64, :],
                                                    bqk_sb[0:64, mt:mt + 1])
                        stg = ast.tile([128, QC], F16, tag="astg")
                        nc.vector.tensor_scalar_add(stg[64:128, :], ps[64:128, :],
                                                    bqk_sb[64:128, mt:mt + 1])
                        nc.sync.dma_start(hi_dst, stg[64:128, :])
                    for sub in range(4):
                        psv = apsV.tile([128, 192], F32, tag="psV",
                                        name=f"psV{tch}_{sub}")
                        for cch in range(NCCH):
                            nc.tensor.matmul(
                                psv[:], xslab[:, cch, sub * 128:(sub + 1) * 128],
                                wv_sb[:, cch, :],
                                start=(cch == 0), stop=(cch == NCCH - 1))
                        tt = tch * 4 + sub
                        for h in range(HPC):
                            nc.vector.tensor_copy(
                                vones[:, tt, V_DST[h]:V_DST[h] + 64],
                                psv[:, h * 64:(h + 1) * 64])

            # ---------------- Phase B: attention ----------------
            with (
                nc.named_scope("phaseB"),
                tc.tile_pool(name="bpsS", bufs=3, space="PSUM") as bpsS,
                tc.tile_pool(name="bpsY", bufs=2, space="PSUM") as bpsY,
                tc.tile_pool(name="bexp", bufs=5) as bexp,
                tc.tile_pool(name="bst", bufs=6) as bst,
            ):
                for jq in range(NJQ):
                    for h in range(HPC):
                        kTh = kT[0:64, h, :]
                        qTh = qT[0:64, h, :]
                        # units of two kt blocks; the last two units are the
                        # diagonal straddles with shrinking widths.
                        units = [("full", (2 * p, 2 * p + 1)) for p in range(2 * jq)]
                        units += [("diag", (4 * jq, 4 * jq + 1)),
                                  ("diag", (4 * jq + 2, 4 * jq + 3))]
                        nu = len(units)
                        es_info = [None] * nu
                        psY = bpsY.tile([128, QC], F32, tag="psY")

                        def emit_S(ui):
                            kind, kts = units[ui]
                            ps = bpsS.tile([128, 1024], F32, tag="psS")
                            es = bexp.tile([128, 1024], F16, tag="es")
                            offs = []
                            pos = 0
                            for kt in kts:
                                r = kt - 4 * jq
                                off = max(r, 0) * KTS
                                w = QC - off
                                nc.tensor.matmul(
                                    ps[:, pos:pos + w],
                                    kTh[:, kt * KTS:(kt + 1) * KTS],
                                    qTh[:, jq * QC + off:(jq + 1) * QC],
                                    start=True, stop=True)
                                offs.append((kt, off, w, pos))
                                pos += w
                            nc.scalar.activation(es[:, 0:pos], ps[:, 0:pos], AF.Exp,
                                                 bias=shift_sb[:, 0:1])
                            if kind == "diag":
                                for kt, off, w, pos_ in offs:
                                    nc.vector.tensor_mul(
                                        es[:, pos_:pos_ + 128],
                                        es[:, pos_:pos_ + 128], tri_sb[:])
                            es_info[ui] = (es, offs)

                        def emit_AV(ui):
                            es, offs = es_info[ui]
                            vc = V_LHST[h]
                            for j, (kt, off, w, pos_) in enumerate(offs):
                                nc.tensor.matmul(
                                    psY[:, off:QC],
                                    vones[:, kt, vc:vc + 128],
                                    es[:, pos_:pos_ + w],
                                    start=(ui == 0 and j == 0),
                                    stop=(ui == nu - 1 and j == len(offs) - 1))
                            es_info[ui] = None

                        # software pipeline: S-runs and AV-runs batched 2 units
                        emit_S(0)
                        if nu > 1:
                            emit_S(1)
                        for ui in range(nu):
                            if ui % 2 == 0:
                                if ui + 2 < nu:
                                    emit_S(ui + 2)
                                if ui + 3 < nu:
                                    emit_S(ui + 3)
                            emit_AV(ui)

                        # normalize: stage both psY halves to SBUF (frees the
                        # bank fast), reciprocal + partition-shift DMA + mult.
                        ystage = bst.tile([128, QC], F32, tag="ystage")
                        rstage = bst.tile([128, QC], F32, tag="rstage")
                        rt = bst.tile([128, QC], F32, tag="rt")
                        ycols = slice(jq * QC, (jq + 1) * QC)
                        if h == 1:  # [1|V]: rowsum on 0:64, y on 64:128
                            nc.vector.tensor_copy(ystage[64:128, :], psY[64:128, :])
                            nc.vector.tensor_copy(rstage[0:64, :], psY[0:64, :])
                            nc.vector.reciprocal(rstage[0:64, :], rstage[0:64, :])
                            nc.sync.dma_start(rt[64:128, :], rstage[0:64, :])
                            nc.vector.tensor_mul(yT0[64:128, ycols],
                                                 ystage[64:128, :], rt[64:128, :])
                        else:       # [V|1]: y on 0:64, rowsum on 64:128
                            nc.vector.tensor_copy(ystage[0:64, :], psY[0:64, :])
                            nc.vector.tensor_copy(rstage[64:128, :], psY[64:128, :])
                            nc.vector.reciprocal(rstage[64:128, :], rstage[64:128, :])
                            nc.sync.dma_start(rt[0:64, :], rstage[64:128, :])
                            dst = yT0[0:64, ycols] if h == 0 else yT1[:, ycols]
                            nc.vector.tensor_mul(dst, ystage[0:64, :], rt[0:64, :])

            if debug:
                nc.sync.dma_start(dq, qT[:])
                nc.sync.dma_start(dk, kT[:])
                nc.sync.dma_start(dv, vones[:])
                nc.sync.dma_start(dy0, yT0[:])
                nc.sync.dma_start(dy1, yT1[:])

            # ---------------- Phase D: output projection ----------------
            with (
                nc.named_scope("phaseD"),
                tc.tile_pool(name="dw", bufs=1) as dw,
                tc.tile_pool(name="dz", bufs=3) as dz,
                tc.tile_pool(name="dps", bufs=3, space="PSUM") as dps,
            ):
                wp0_sb = dw.tile([128, C], F16, tag="wp0")
                wp1_sb = dw.tile([64, C], F16, tag="wp1")
                nc.sync.dma_start(wp0_sb[:], wp[0:128, :])
                nc.sync.dma_start(wp1_sb[:], wp[128:192, :])

                for tt in range(NTT):
                    tsl = slice(tt * 128, (tt + 1) * 128)
                    pz = dps.tile([128, C], F32, tag="pz")
                    for (a, b) in ((0, 512), (512, 768)):
                        nc.tensor.matmul(pz[:, a:b], yT0[:, tsl], wp0_sb[:, a:b],
                                         start=True, stop=False)
                        nc.tensor.matmul(pz[:, a:b], yT1[:, tsl], wp1_sb[:, a:b],
                                         start=False, stop=True)
                    zt = dz.tile([128, C], F32, tag="zt")
                    nc.vector.tensor_copy(zt[:], pz[:])
                    nc.sync.dma_start(z[tsl, :], zt[:])

    nc.compile()
    return nc


def _get_program():
    if "nc" not in _cache:
        _cache["nc"] = _build()
    return _cache["nc"]


def kernel(x, W_attn, b_attn, W_proj, b_proj):
    global last_results
    from concourse.bass_utils import run_bass_kernel_spmd

    x = np.asarray(x, dtype=np.float32)
    W_attn = np.asarray(W_attn, dtype=np.float32)
    b_attn = np.asarray(b_attn, dtype=np.float32)
    W_proj = np.asarray(W_proj, dtype=np.float32)
    b_proj = np.asarray(b_proj, dtype=np.float32)

    Wq, Wk, Wv = W_attn[:, 0:C], W_attn[:, C:2 * C], W_attn[:, 2 * C:3 * C]
    bq, bk, bv = b_attn[0:C], b_attn[C:2 * C], b_attn[2 * C:3 * C]
    scale = 1.0 / np.sqrt(D)

    xTb = [np.ascontiguousarray(x[b].T).astype(np.float16) for b in range(B)]
    tri = np.triu(np.ones((128, 128), dtype=np.float16))  # keep f >= p

    in_maps = []
    for core in range(NCORE):
        b = core // 4
        h0 = 3 * (core % 4)
        cs = slice(h0 * D, (h0 + HPC) * D)  # this core's 192 channels
        q_w = Wq[:, cs] * scale
        k_w = Wk[:, cs]
        # columns: [q0 q1 | k0 k1 | q2 k2]
        wqk_i = np.concatenate(
            [q_w[:, 0:128], k_w[:, 0:128], q_w[:, 128:192], k_w[:, 128:192]],
            axis=1)
        bq_c = bq[cs] * scale
        bk_c = bk[cs]
        bqk_i = np.zeros((128, 3), dtype=np.float32)
        bqk_i[:, 0] = bq_c[0:128]
        bqk_i[:, 1] = bk_c[0:128]
        bqk_i[0:64, 2] = bq_c[128:192]
        bqk_i[64:128, 2] = bk_c[128:192]
        in_maps.append({
            "xT": xTb[b],
            "wqk": wqk_i.astype(np.float16),
            "wv": np.ascontiguousarray(Wv[:, cs]).astype(np.float16),
            "wp": np.ascontiguousarray(W_proj[cs, :]).astype(np.float16),
            "bqk": bqk_i,
            "trimask": tri,
        })

    nc = _get_program()
    trace = os.environ.get("CC_ATTN_TRACE", "0") == "1"
    res = run_bass_kernel_spmd(nc, in_maps, core_ids=list(range(NCORE)),
                               trace=trace)
    last_results = res

    bias_row = (b_proj + bv @ W_proj).astype(np.float32)  # [768]
    out = np.empty((B, T, C), dtype=np.float32)
    for b in range(B):
        acc = res.results[4 * b]["z"].astype(np.float32).copy()
        for g in range(1, 4):
            acc += res.results[4 * b + g]["z"]
        out[b] = acc + bias_row
    return out
